# revision 23
# baseline (speedup 1.0000x reference)
"""Masked-softmax attention on 8 trn2 NeuronCores.

Reference computation (per batch b):
    att = q @ k                        # [n_q, n_k], k given pre-transposed [d, n_k]
    att = where(mask==0, -1e9, att)
    att = softmax(att, -1) / sqrt(d)
    out = (att @ v).T                  # returned [n_dv, n_q]

Sharding: data-parallel over batch: B=16 -> 2 batches per core x 8 cores.

Host-side, per batch, the key dimension is COMPACTED: masked-out keys
contribute exactly 0 to both the softmax numerator and denominator (the
reference's exp(-1e9 - anything) underflows to +0.0 in fp32), so we gather
only the unmasked columns of k / rows of v, padded up to a multiple of 128
(padding killed by the same -1e9 bias). With a Bernoulli(0.5) mask this
halves the contraction length. Exact, not an approximation.

Device-side plan (per batch, all matmuls in float32r = full-rate PE):
    - Work in the TRANSPOSED score layout S^T[k, q] (k on partitions):
        S^T tile [128k, 512q] = k_slice[d,128k]^T @ qT[d, 512q]  (2 d-chunk accum)
      `k` input [d, n_k] is directly the stationary operand; `q` is transposed
      host-side during sharding so qT[d, n_q] is directly the moving operand.
    - softmax is shift-invariant, so instead of the row max we subtract a
      CONSTANT shift (scores ~ N(0, d) with d=256 -> |s| < ~110 always;
      exp(s-shift) can't overflow and dominant terms can't underflow).
      Mask + shift fold into the scalar-engine exp as a per-partition bias:
        e[k, q] = exp(s + bias_k),  bias_k = -shift - 1e9*(1-mask_k)
    - out^T[dv, q] += v_tile[128k, dv_chunk]^T @ e   (v is directly stationary)
      z[dv, q]    += sixteens[128k, 128]^T @ e       (= 16Z in EVERY partition:
      the all-16s stationary matrix computes the row sum AND broadcasts it,
      folding in the post-softmax 1/sqrt(d)=1/16 scale)
    - out = out^T * (1/z) (DVE approx reciprocal) -> [dv, n_q], the required
      output layout.

Input DMAs alternate between the SP and Activation HWDGE queues (one queue
per issuing engine) so transfers parallelize; the mask rides the gpsimd
SWDGE so the bias is ready immediately.
"""

import numpy as np

import concourse.bacc as bacc
import concourse.mybir as mybir
import concourse.tile as tile
from concourse.bass_utils import run_bass_kernel_spmd

P = 128          # partitions
D = 256          # d == n_dv
S = 2048         # n_q
NB = 2           # batches per core
QS = 512         # q-stripe width (max fp32 matmul moving dim)
NQS = S // QS    # 4 q-stripes
NCORES = 8
SHIFT = 60.0     # constant softmax shift (see module docstring)

F32 = mybir.dt.float32
F32R = mybir.dt.float32r
I32 = mybir.dt.int32
EXP = mybir.ActivationFunctionType.Exp
MULT = mybir.AluOpType.mult
ADD = mybir.AluOpType.add


def build(sk):
    """Build the per-core program. sk = compacted key length (mult of 128)."""
    from contextlib import ExitStack

    nkt = sk // P  # number of k-tiles
    nc = bacc.Bacc()
    qT = nc.declare_dram_parameter("qT", [NB, D, S], F32R, isOutput=False)
    kk = nc.declare_dram_parameter("k", [NB, D, sk], F32R, isOutput=False)
    vv = nc.declare_dram_parameter("v", [NB, sk, D], F32R, isOutput=False)
    bb = nc.declare_dram_parameter("bias", [NB, P, sk // P], F32, isOutput=False)
    out = nc.declare_dram_parameter("out", [NB, D, S], F32, isOutput=True)

    with tile.TileContext(nc) as tc, ExitStack() as ctx:
        consts = ctx.enter_context(tc.tile_pool(name="consts", bufs=1))
        inp = ctx.enter_context(tc.tile_pool(name="inp", bufs=2))
        epool = ctx.enter_context(tc.tile_pool(name="e", bufs=4))
        opool = ctx.enter_context(tc.tile_pool(name="o", bufs=2))
        zpool = ctx.enter_context(tc.tile_pool(name="z", bufs=2))
        ps_s = ctx.enter_context(tc.tile_pool(name="ps_s", bufs=3, space="PSUM"))
        ps_o = ctx.enter_context(tc.tile_pool(name="ps_o", bufs=2, space="PSUM"))
        ps_z = ctx.enter_context(tc.tile_pool(name="ps_z", bufs=1, space="PSUM"))

        # (memset can't emit f32r; stage in f32 and DVE-copy to round)
        sixteens_f = consts.tile([P, P], F32)
        nc.vector.memset(sixteens_f, 16.0)
        sixteens = consts.tile([P, P], F32R)
        nc.vector.tensor_copy(sixteens, sixteens_f)

        # Warmup Exp: walrus attaches the implicit ACT table load to the
        # first Exp, which eats its sync-wait slots; give it a dep-free one
        # (also hides the ~2.7us table load under the input DMA fill).
        warm_in = consts.tile([P, 1], F32)
        nc.vector.memset(warm_in, 0.0)
        warm_out = consts.tile([P, 1], F32)
        nc.scalar.activation(warm_out, warm_in, EXP)

        # PE warmup: dep-free matmuls during the initial DMA fill so the HAM
        # clock gate reaches K=8/8 before the real matmuls start.
        for w in range(14):
            wp = ps_s.tile([P, P], F32, tag="s", name=f"warm{w}")
            nc.tensor.matmul(wp, lhsT=sixteens, rhs=sixteens, start=True, stop=True)

        for b in range(NB):
            # ---- load inputs (double-buffered across batches). Emission
            # order = consumption order; alternate HWDGE queues.
            kts = [inp.tile([P, sk], F32R, tag=f"k{c}", name=f"kt{c}") for c in range(2)]
            qts = [inp.tile([P, S], F32R, tag=f"q{c}", name=f"qt{c}") for c in range(2)]
            vt_all = inp.tile([P, nkt, D], F32R, tag="v", name="vt_all")
            vts = [vt_all[:, t, :] for t in range(nkt)]


            # Queue assignment: gpsimd SWDGE carries the whole k matrix and
            # v in big blocks; sync (SP HWDGE) carries bias + all q stripes;
            # the ACT engine issues nothing for inputs so exp work never
            # delays critical input.
            for c in range(2):  # whole k d-half per SWDGE block
                nc.gpsimd.dma_start(
                    out=kts[c], in_=kk[b, c * P : (c + 1) * P, :]
                )
            vh = (nkt + 1) // 2
            for t0, t1 in ((0, vh), (vh, nkt)):  # v in 2 big SWDGE blocks
                if t1 > t0:
                    nc.gpsimd.dma_start(
                        out=vt_all[:, t0:t1, :],
                        in_=vv[b, t0 * P : t1 * P, :].rearrange(
                            "(t p) d -> p t d", p=P
                        ),
                    )
            biast = inp.tile([P, nkt], F32, tag="bias")
            nc.sync.dma_start(out=biast, in_=bb[b])
            for j in range(NQS):  # q stripes via SP HWDGE, stripe 0 first
                for c in range(2):
                    nc.sync.dma_start(
                        out=qts[c][:, j * QS : (j + 1) * QS],
                        in_=qT[b, c * P : (c + 1) * P, j * QS : (j + 1) * QS],
                    )


            # ---- compute, one 512-wide q-stripe at a time
            for s in range(NQS):
                qoff, qw = s * QS, QS
                qsl = slice(qoff, qoff + qw)
                op0 = ps_o.tile([P, QS], F32, tag="o0", name="op0")[:, :qw]
                op1 = ps_o.tile([P, QS], F32, tag="o1", name="op1")[:, :qw]
                zp = ps_z.tile([P, QS], F32, tag="z", name="zp")[:, :qw]
                # e-tiles accumulated on DVE per Z matmul. On the very last
                # stripe the DVE add-chain would BE the kernel tail (PE is
                # idle by then), so Z goes straight to the PE there.
                QUAD = 1 if (b == NB - 1 and s == NQS - 1) else 4
                acc_e = None
                nacc = 0
                nzmm = (nkt + QUAD - 1) // QUAD
                zi = 0
                for t in range(nkt):
                    ksl = slice(t * P, (t + 1) * P)
                    sp = ps_s.tile([P, QS], F32, tag="s", name="sp")[:, :qw]
                    nc.tensor.matmul(
                        sp, lhsT=kts[0][:, ksl], rhs=qts[0][:, qsl],
                        start=True, stop=False,
                    )
                    nc.tensor.matmul(
                        sp, lhsT=kts[1][:, ksl], rhs=qts[1][:, qsl],
                        start=False, stop=True,
                    )
                    e = epool.tile([P, QS], F32R, tag="e", name="e")[:, :qw]
                    nc.scalar.activation(e, sp, EXP, bias=biast[:, t : t + 1])
                    first, last = t == 0, t == nkt - 1
                    nc.tensor.matmul(
                        op0, lhsT=vts[t][:, 0:P], rhs=e, start=first, stop=last,
                    )
                    nc.tensor.matmul(
                        op1, lhsT=vts[t][:, P : 2 * P], rhs=e, start=first, stop=last,
                    )
                    # Z: a running DVE accumulator sums QUAD e-tiles so only
                    # ceil(nkt/QUAD) Z matmuls run (PE cycles -> idle DVE)
                    if acc_e is None:
                        acc_e, nacc = e, 1
                    else:
                        na = epool.tile([P, QS], F32R, tag="ep", name="na")[:, :qw]
                        nc.vector.tensor_tensor(na, acc_e, e, ADD)
                        acc_e = na
                        nacc += 1
                    if nacc == QUAD or t == nkt - 1:
                        nc.tensor.matmul(
                            zp, lhsT=sixteens, rhs=acc_e,
                            start=zi == 0, stop=zi == nzmm - 1,
                        )
                        zi += 1
                        acc_e, nacc = None, 0
                # normalize: out = out_unnorm * (1/(16Z)); zp already holds
                # 16Z in every partition. ~18-bit reciprocal, 5x faster than
                # exact; z is far from denorm/inf so approx edge cases can't
                # hit. Processed in halves so the tail (recip -> mult -> DMA)
                # pipelines.
                zbs = zpool.tile([P, QS], F32, tag="zbs", name="zbs")[:, :qw]
                o0 = opool.tile([P, QS], F32, tag="so0", name="o0")[:, :qw]
                o1 = opool.tile([P, QS], F32, tag="so1", name="o1")[:, :qw]
                for h in range(2):
                    hs = slice(h * (qw // 2), (h + 1) * (qw // 2))
                    oqsl = slice(qoff + h * (qw // 2), qoff + (h + 1) * (qw // 2))
                    nc.vector.reciprocal_approx_fast(out=zbs[:, hs], in_=zp[:, hs])
                    nc.vector.tensor_tensor(o0[:, hs], op0[:, hs], zbs[:, hs], MULT)
                    nc.vector.tensor_tensor(o1[:, hs], op1[:, hs], zbs[:, hs], MULT)
                    nc.sync.dma_start(out=out[b, 0:P, oqsl], in_=o0[:, hs])
                    nc.scalar.dma_start(out=out[b, P : 2 * P, oqsl], in_=o1[:, hs])

    return nc


def make_in_maps(q, k, v, mask):
    """Shard over batch; transpose q; compact the key dim to unmasked keys."""
    q = np.asarray(q, dtype=np.float32)
    k = np.asarray(k, dtype=np.float32)
    v = np.asarray(v, dtype=np.float32)
    mask = np.asarray(mask, dtype=np.int32).reshape(len(q), -1)

    B = len(q)
    idxs = [np.nonzero(mask[b])[0] for b in range(B)]
    n_eff = max((len(ix) for ix in idxs), default=1)
    sk = max(P, ((n_eff + P - 1) // P) * P)  # padded compacted key length

    kg = np.zeros((B, D, sk), dtype=np.float32)
    vg = np.zeros((B, sk, D), dtype=np.float32)
    # exp bias: -SHIFT for real keys, -1e9 for padding (kills it exactly),
    # laid out [P, sk//P] partition-major to match the k-tile slicing
    bg = np.full((B, sk), -1.0e9, dtype=np.float32)
    for b in range(B):
        ix = idxs[b]
        kg[b, :, : len(ix)] = k[b][:, ix]
        vg[b, : len(ix)] = v[b][ix]
        bg[b, : len(ix)] = -SHIFT
    bgt = np.ascontiguousarray(
        bg.reshape(B, sk // P, P).transpose(0, 2, 1)
    )  # [B, P, nkt]

    in_maps = []
    for i in range(NCORES):
        sl = slice(i * NB, (i + 1) * NB)
        in_maps.append(
            {
                "qT": np.ascontiguousarray(np.transpose(q[sl], (0, 2, 1))),
                "k": np.ascontiguousarray(kg[sl]),
                "v": np.ascontiguousarray(vg[sl]),
                "bias": np.ascontiguousarray(bgt[sl]),
            }
        )
    return in_maps, sk


def run(q, k, v, mask, **kwargs):
    in_maps, sk = make_in_maps(q, k, v, mask)
    nc = build(sk)
    nc.finalize()  # run the Bacc pass pipeline (reg alloc, wait splitting)
    res = run_bass_kernel_spmd(nc, in_maps, list(range(NCORES)), **kwargs)
    out = np.concatenate([r["out"] for r in res.results], axis=0)
    return out, res


def kernel(q, k, v, mask):
    out, _ = run(q, k, v, mask)
    return out


# revision 24
# speedup vs baseline: 1.0109x; 1.0109x over previous
"""Masked-softmax attention on 8 trn2 NeuronCores.

Reference computation (per batch b):
    att = q @ k                        # [n_q, n_k], k given pre-transposed [d, n_k]
    att = where(mask==0, -1e9, att)
    att = softmax(att, -1) / sqrt(d)
    out = (att @ v).T                  # returned [n_dv, n_q]

Sharding: data-parallel over batch: B=16 -> 2 batches per core x 8 cores.

Host-side, per batch, the key dimension is COMPACTED: masked-out keys
contribute exactly 0 to both the softmax numerator and denominator (the
reference's exp(-1e9 - anything) underflows to +0.0 in fp32), so we gather
only the unmasked columns of k / rows of v, padded up to a multiple of 128
(padding killed by the same -1e9 bias). With a Bernoulli(0.5) mask this
halves the contraction length. Exact, not an approximation.

Device-side plan (per batch, all matmuls in float32r = full-rate PE):
    - Work in the TRANSPOSED score layout S^T[k, q] (k on partitions):
        S^T tile [128k, 512q] = k_slice[d,128k]^T @ qT[d, 512q]  (2 d-chunk accum)
      `k` input [d, n_k] is directly the stationary operand; `q` is transposed
      host-side during sharding so qT[d, n_q] is directly the moving operand.
    - softmax is shift-invariant, so instead of the row max we subtract a
      CONSTANT shift (scores ~ N(0, d) with d=256 -> |s| < ~110 always;
      exp(s-shift) can't overflow and dominant terms can't underflow).
      Mask + shift fold into the scalar-engine exp as a per-partition bias:
        e[k, q] = exp(s + bias_k),  bias_k = -shift - 1e9*(1-mask_k)
    - out^T[dv, q] += v_tile[128k, dv_chunk]^T @ e   (v is directly stationary)
      z[dv, q]    += sixteens[128k, 128]^T @ e       (= 16Z in EVERY partition:
      the all-16s stationary matrix computes the row sum AND broadcasts it,
      folding in the post-softmax 1/sqrt(d)=1/16 scale)
    - out = out^T * (1/z) (DVE approx reciprocal) -> [dv, n_q], the required
      output layout.

Input DMAs alternate between the SP and Activation HWDGE queues (one queue
per issuing engine) so transfers parallelize; the mask rides the gpsimd
SWDGE so the bias is ready immediately.
"""

import numpy as np

import concourse.bacc as bacc
import concourse.mybir as mybir
import concourse.tile as tile
from concourse.bass_utils import run_bass_kernel_spmd

P = 128          # partitions
D = 256          # d == n_dv
S = 2048         # n_q
NB = 2           # batches per core
QS = 512         # q-stripe width (max fp32 matmul moving dim)
NQS = S // QS    # 4 q-stripes
NCORES = 8
SHIFT = 60.0     # constant softmax shift (see module docstring)

F32 = mybir.dt.float32
F32R = mybir.dt.float32r
I32 = mybir.dt.int32
EXP = mybir.ActivationFunctionType.Exp
MULT = mybir.AluOpType.mult
ADD = mybir.AluOpType.add


def build(sk):
    """Build the per-core program. sk = compacted key length (mult of 128)."""
    from contextlib import ExitStack

    nkt = sk // P  # number of k-tiles
    nc = bacc.Bacc()
    qT = nc.declare_dram_parameter("qT", [NB, D, S], F32R, isOutput=False)
    kk = nc.declare_dram_parameter("k", [NB, D, sk], F32R, isOutput=False)
    vv = nc.declare_dram_parameter("v", [NB, sk, D], F32R, isOutput=False)
    bb = nc.declare_dram_parameter("bias", [NB, P, sk // P], F32, isOutput=False)
    out = nc.declare_dram_parameter("out", [NB, D, S], F32, isOutput=True)

    with tile.TileContext(nc) as tc, ExitStack() as ctx:
        consts = ctx.enter_context(tc.tile_pool(name="consts", bufs=1))
        inp = ctx.enter_context(tc.tile_pool(name="inp", bufs=2))
        epool = ctx.enter_context(tc.tile_pool(name="e", bufs=4))
        opool = ctx.enter_context(tc.tile_pool(name="o", bufs=2))
        zpool = ctx.enter_context(tc.tile_pool(name="z", bufs=2))
        ps_s = ctx.enter_context(tc.tile_pool(name="ps_s", bufs=3, space="PSUM"))
        ps_o = ctx.enter_context(tc.tile_pool(name="ps_o", bufs=2, space="PSUM"))
        ps_z = ctx.enter_context(tc.tile_pool(name="ps_z", bufs=1, space="PSUM"))

        # (memset can't emit f32r; stage in f32 and DVE-copy to round)
        sixteens_f = consts.tile([P, P], F32)
        nc.vector.memset(sixteens_f, 16.0)
        sixteens = consts.tile([P, P], F32R)
        nc.vector.tensor_copy(sixteens, sixteens_f)

        # Warmup Exp: walrus attaches the implicit ACT table load to the
        # first Exp, which eats its sync-wait slots; give it a dep-free one
        # (also hides the ~2.7us table load under the input DMA fill).
        warm_in = consts.tile([P, 1], F32)
        nc.vector.memset(warm_in, 0.0)
        warm_out = consts.tile([P, 1], F32)
        nc.scalar.activation(warm_out, warm_in, EXP)

        # PE warmup: dep-free matmuls during the initial DMA fill so the HAM
        # clock gate reaches K=8/8 before the real matmuls start.
        for w in range(14):
            wp = ps_s.tile([P, P], F32, tag="s", name=f"warm{w}")
            nc.tensor.matmul(wp, lhsT=sixteens, rhs=sixteens, start=True, stop=True)

        for b in range(NB):
            # ---- load inputs (double-buffered across batches). Emission
            # order = consumption order; alternate HWDGE queues.
            kts = [inp.tile([P, sk], F32R, tag=f"k{c}", name=f"kt{c}") for c in range(2)]
            qts = [inp.tile([P, S], F32R, tag=f"q{c}", name=f"qt{c}") for c in range(2)]
            vt_all = inp.tile([P, nkt, D], F32R, tag="v", name="vt_all")
            vts = [vt_all[:, t, :] for t in range(nkt)]


            # Queue assignment: gpsimd SWDGE carries the whole k matrix and
            # v in big blocks; sync (SP HWDGE) carries bias + all q stripes;
            # the ACT engine issues nothing for inputs so exp work never
            # delays critical input.
            for c in range(2):  # whole k d-half per SWDGE block
                nc.gpsimd.dma_start(
                    out=kts[c], in_=kk[b, c * P : (c + 1) * P, :]
                )
            vh = (nkt + 1) // 2
            for t0, t1 in ((0, vh), (vh, nkt)):  # v in 2 big SWDGE blocks
                if t1 > t0:
                    nc.gpsimd.dma_start(
                        out=vt_all[:, t0:t1, :],
                        in_=vv[b, t0 * P : t1 * P, :].rearrange(
                            "(t p) d -> p t d", p=P
                        ),
                    )
            biast = inp.tile([P, nkt], F32, tag="bias")
            nc.sync.dma_start(out=biast, in_=bb[b])
            for j in range(NQS):  # q stripes via SP HWDGE, stripe 0 first
                for c in range(2):
                    nc.sync.dma_start(
                        out=qts[c][:, j * QS : (j + 1) * QS],
                        in_=qT[b, c * P : (c + 1) * P, j * QS : (j + 1) * QS],
                    )


            # ---- compute, one 512-wide q-stripe at a time
            for s in range(NQS):
                qoff, qw = s * QS, QS
                qsl = slice(qoff, qoff + qw)
                op0 = ps_o.tile([P, QS], F32, tag="o0", name="op0")[:, :qw]
                op1 = ps_o.tile([P, QS], F32, tag="o1", name="op1")[:, :qw]
                zp = ps_z.tile([P, QS], F32, tag="z", name="zp")[:, :qw]
                QUAD = 4  # e-tiles accumulated on DVE per Z matmul
                acc_e = None
                nacc = 0
                nzmm = (nkt + QUAD - 1) // QUAD
                zi = 0
                for t in range(nkt):
                    ksl = slice(t * P, (t + 1) * P)
                    sp = ps_s.tile([P, QS], F32, tag="s", name="sp")[:, :qw]
                    nc.tensor.matmul(
                        sp, lhsT=kts[0][:, ksl], rhs=qts[0][:, qsl],
                        start=True, stop=False,
                    )
                    nc.tensor.matmul(
                        sp, lhsT=kts[1][:, ksl], rhs=qts[1][:, qsl],
                        start=False, stop=True,
                    )
                    e = epool.tile([P, QS], F32R, tag="e", name="e")[:, :qw]
                    nc.scalar.activation(e, sp, EXP, bias=biast[:, t : t + 1])
                    first, last = t == 0, t == nkt - 1
                    nc.tensor.matmul(
                        op0, lhsT=vts[t][:, 0:P], rhs=e, start=first, stop=last,
                    )
                    nc.tensor.matmul(
                        op1, lhsT=vts[t][:, P : 2 * P], rhs=e, start=first, stop=last,
                    )
                    # Z: a running DVE accumulator sums QUAD e-tiles so only
                    # ceil(nkt/QUAD) Z matmuls run (PE cycles -> idle DVE)
                    if acc_e is None:
                        acc_e, nacc = e, 1
                    else:
                        na = epool.tile([P, QS], F32R, tag="ep", name="na")[:, :qw]
                        nc.vector.tensor_tensor(na, acc_e, e, ADD)
                        acc_e = na
                        nacc += 1
                    if nacc == QUAD or t == nkt - 1:
                        nc.tensor.matmul(
                            zp, lhsT=sixteens, rhs=acc_e,
                            start=zi == 0, stop=zi == nzmm - 1,
                        )
                        zi += 1
                        acc_e, nacc = None, 0
                # normalize: out = out_unnorm * (1/(16Z)); zp already holds
                # 16Z in every partition. ~18-bit reciprocal, 5x faster than
                # exact; z is far from denorm/inf so approx edge cases can't
                # hit. Processed in halves so the tail (recip -> mult -> DMA)
                # pipelines.
                zbs = zpool.tile([P, QS], F32, tag="zbs", name="zbs")[:, :qw]
                o0 = opool.tile([P, QS], F32, tag="so0", name="o0")[:, :qw]
                o1 = opool.tile([P, QS], F32, tag="so1", name="o1")[:, :qw]
                for h in range(2):
                    hs = slice(h * (qw // 2), (h + 1) * (qw // 2))
                    oqsl = slice(qoff + h * (qw // 2), qoff + (h + 1) * (qw // 2))
                    nc.vector.reciprocal_approx_fast(out=zbs[:, hs], in_=zp[:, hs])
                    nc.vector.tensor_tensor(o0[:, hs], op0[:, hs], zbs[:, hs], MULT)
                    nc.vector.tensor_tensor(o1[:, hs], op1[:, hs], zbs[:, hs], MULT)
                    nc.sync.dma_start(out=out[b, 0:P, oqsl], in_=o0[:, hs])
                    nc.scalar.dma_start(out=out[b, P : 2 * P, oqsl], in_=o1[:, hs])

    return nc


def make_in_maps(q, k, v, mask):
    """Shard over batch; transpose q; compact the key dim to unmasked keys."""
    q = np.asarray(q, dtype=np.float32)
    k = np.asarray(k, dtype=np.float32)
    v = np.asarray(v, dtype=np.float32)
    mask = np.asarray(mask, dtype=np.int32).reshape(len(q), -1)

    B = len(q)
    idxs = [np.nonzero(mask[b])[0] for b in range(B)]
    n_eff = max((len(ix) for ix in idxs), default=1)
    sk = max(P, ((n_eff + P - 1) // P) * P)  # padded compacted key length

    kg = np.zeros((B, D, sk), dtype=np.float32)
    vg = np.zeros((B, sk, D), dtype=np.float32)
    # exp bias: -SHIFT for real keys, -1e9 for padding (kills it exactly),
    # laid out [P, sk//P] partition-major to match the k-tile slicing
    bg = np.full((B, sk), -1.0e9, dtype=np.float32)
    for b in range(B):
        ix = idxs[b]
        kg[b, :, : len(ix)] = k[b][:, ix]
        vg[b, : len(ix)] = v[b][ix]
        bg[b, : len(ix)] = -SHIFT
    bgt = np.ascontiguousarray(
        bg.reshape(B, sk // P, P).transpose(0, 2, 1)
    )  # [B, P, nkt]

    in_maps = []
    for i in range(NCORES):
        sl = slice(i * NB, (i + 1) * NB)
        in_maps.append(
            {
                "qT": np.ascontiguousarray(np.transpose(q[sl], (0, 2, 1))),
                "k": np.ascontiguousarray(kg[sl]),
                "v": np.ascontiguousarray(vg[sl]),
                "bias": np.ascontiguousarray(bgt[sl]),
            }
        )
    return in_maps, sk


def run(q, k, v, mask, **kwargs):
    in_maps, sk = make_in_maps(q, k, v, mask)
    nc = build(sk)
    nc.finalize()  # run the Bacc pass pipeline (reg alloc, wait splitting)
    res = run_bass_kernel_spmd(nc, in_maps, list(range(NCORES)), **kwargs)
    out = np.concatenate([r["out"] for r in res.results], axis=0)
    return out, res


def kernel(q, k, v, mask):
    out, _ = run(q, k, v, mask)
    return out
